# revision 1
# baseline (speedup 1.0000x reference)
"""Multi-head self-attention (QK^T -> softmax -> ctx -> linear) on 8 TRN2 cores.

Sharding: each core owns one (batch, query-block) shard: batch = core//4,
queries [qlo, qlo+512) with qlo = (core%4)*512. Attention needs all keys of
the core's batch, so keys are replicated per batch; no collectives needed.

Math per core (head h, its 512 queries q, all 2048 keys k):
  S_T[k, q]   = sum_d x[k, h*64+d] * x[q, h*64+d]          (PE, f32r)
  P_T[k, q]   = exp(0.125 * S_T[k, q])                     (ACT, PSUM->SBUF)
  ctxT[m, q]  = sum_k xa[k, m] * P_T[k, q]                 (PE, accumulate)
     where xa = [x | ones] so row m=64 is the softmax denominator
  chunk[i, q] = ctxT[d, q] / ctxT[64, q]    (i = h*64+d)   (DVE)
  out[q, o]   = sum_i chunk[i, q]*W[o, i] + b[o]           (PE; bias via K=1 mm)

Everything stays in the transposed orientation so no on-chip transposes of the
attention matrix are ever needed, and ctxT directly feeds the final matmul as
the stationary operand.
"""

import sys

for _p in ("/opt/trn_rl_repo", "/root/.axon_site/_ro/trn_rl_repo"):
    if _p not in sys.path:
        sys.path.append(_p)

import numpy as np

import concourse.bacc as bacc
import concourse.bass as bass
import concourse.library_config as library_config
import concourse.mybir as mybir
import concourse.tile as tile

F32 = mybir.dt.float32
F32R = mybir.dt.float32r

B, L, H, NH, DH = 2, 2048, 1024, 16, 64
NCORES = 8


def build_nc(L=2048, NH=16, DH=64, QB=512, H=1024, use_f32r=True):
    """One SPMD Bass program; per-core data differences live in the inputs."""
    KBLKS = L // 128           # key blocks of 128
    NPAIR = NH // 2            # head pairs (2 heads = 128 partitions)
    AUG = DH + 1               # x augmented with a ones column
    HC = H // 128              # hidden chunks for the final matmul
    OBW = min(512, H)          # output column block width
    OBLKS = H // OBW
    QSUB = QB // 128
    SCALE = float(1.0 / np.sqrt(DH))
    mmdt = F32R if use_f32r else F32

    def r(ap):
        return ap if ap.dtype == mmdt else ap.bitcast(mmdt)

    nc = bacc.Bacc("TRN2")
    xt = nc.declare_dram_parameter("xt", [NPAIR, 128, L], mmdt, isOutput=False)
    xq = nc.declare_dram_parameter("xq", [NPAIR, 128, QB], mmdt, isOutput=False)
    xa = nc.declare_dram_parameter("xa", [NH, 128, KBLKS * AUG], mmdt, isOutput=False)
    wt = nc.declare_dram_parameter("wt", [128, HC * H], mmdt, isOutput=False)
    bias = nc.declare_dram_parameter("bias", [1, H], mmdt, isOutput=False)
    ones = nc.declare_dram_parameter("ones", [1, 128], mmdt, isOutput=False)
    out = nc.declare_dram_parameter("out", [QB, H], F32, isOutput=True)

    with tile.TileContext(nc) as tc:
        with (
            tc.tile_pool(name="xt", bufs=2) as xt_pool,
            tc.tile_pool(name="xq", bufs=2) as xq_pool,
            tc.tile_pool(name="xa", bufs=4) as xa_pool,
            tc.tile_pool(name="p", bufs=4) as p_pool,
            tc.tile_pool(name="consts", bufs=1) as consts,
            tc.tile_pool(name="ctxsb", bufs=NPAIR) as ctx_pool,
            tc.tile_pool(name="recip", bufs=4) as r_pool,
            tc.tile_pool(name="osb", bufs=3) as o_pool,
            tc.tile_pool(name="spsum", bufs=2, space="PSUM") as s_psum,
            tc.tile_pool(name="cpsum", bufs=2, space="PSUM") as c_psum,
        ):
            ones_t = consts.tile([1, 128], mmdt)
            nc.sync.dma_start(ones_t[:], ones[:])
            bias_t = consts.tile([1, H], mmdt)
            nc.sync.dma_start(bias_t[:], bias[:])

            # Pre-broadcast the bias across all 128 partitions once (PE ones
            # matmul) -> bias_bc holds b[o] in every row q.
            bias_bc = consts.tile([128, H], F32)
            for ob in range(OBLKS):
                obsl = slice(ob * OBW, (ob + 1) * OBW)
                bps = s_psum.tile([128, 2 * QB], F32, tag="s")
                nc.tensor.matmul(
                    bps[:, 0:OBW], r(ones_t[:, :]), r(bias_t[0:1, obsl]),
                    start=True, stop=True,
                )
                nc.vector.tensor_copy(bias_bc[:, obsl], bps[:, 0:OBW])

            # Output-projection weights are streamed one hidden-chunk per
            # head pair, so no big DMA ever stalls the attention pipeline.
            wt_ts = [consts.tile([128, H], mmdt, tag=f"wt{c}", name=f"wt{c}") for c in range(HC)]

            # Per-qs output accumulators: each chunk's projection contribution
            # is matmul'd into a briefly-held PSUM slot and DVE-added here,
            # one contribution per kb step of the following pair.
            acc = [consts.tile([128, H], F32, tag=f"acc{q}", name=f"acc{q}") for q in range(QSUB)]

            chunks = []
            contrib_q = []

            def emit_contrib(alt=None):
                c, qs, ob = contrib_q.pop(0)
                qsl = slice(qs * 128, (qs + 1) * 128)
                obsl = slice(ob * OBW, (ob + 1) * OBW)
                tag = ("ctx_a" if (qs + ob) % 2 == 0 else "ctx_b") if alt is None else alt
                cp = c_psum.tile([128, max(QB, OBW)], F32, tag=tag, name=f"cp{c}_{qs}_{ob}")
                nc.tensor.matmul(
                    cp[:, 0:OBW], r(chunks[c][:, qsl]), r(wt_ts[c][:, obsl]),
                    start=True, stop=True,
                )
                prevacc = bias_bc if c == 0 else acc[qs]
                nc.vector.tensor_add(
                    acc[qs][:, obsl], cp[:, 0:OBW], prevacc[:, obsl]
                )

            def emit_norm_p1(ctx_a, ctx_b):
                # Phase 1: reciprocals of the denominator rows + broadcast of
                # head A across DH partitions with a K=1 ones matmul.
                rc_a = r_pool.tile([1, QB], mmdt)
                rc_b = r_pool.tile([1, QB], mmdt)
                with nc.allow_low_precision(reason="f32r rounding for matmul"):
                    nc.vector.reciprocal(rc_a[:], ctx_a[DH : DH + 1, :])
                    nc.vector.reciprocal(rc_b[:], ctx_b[DH : DH + 1, :])
                bc_ps = s_psum.tile([128, 2 * QB], F32, tag="s")
                nc.tensor.matmul(
                    bc_ps[0:DH, 0:QB], r(ones_t[0:1, 0:DH]), rc_a[:],
                    start=True, stop=True,
                )
                return rc_b, bc_ps

            def emit_norm_p2(ctx_a, ctx_b, rc_b, bc_ps):
                # Phase 2 (next kb step, spreading PE load): broadcast head B,
                # then normalize ctxT into the SBUF chunk.
                nc.tensor.matmul(
                    bc_ps[0:DH, QB : 2 * QB], r(ones_t[0:1, 0:DH]), rc_b[:],
                    start=True, stop=True,
                )
                bc_sb = r_pool.tile([128, QB], F32, tag="bc")
                nc.vector.tensor_copy(bc_sb[0:DH, :], bc_ps[0:DH, 0:QB])
                nc.vector.tensor_copy(
                    bc_sb[DH : 2 * DH, :], bc_ps[0:DH, QB : 2 * QB]
                )
                chunk = ctx_pool.tile([128, QB], mmdt)
                nc.vector.tensor_mul(
                    chunk[0:DH, :], ctx_a[0:DH, :], bc_sb[0:DH, :]
                )
                nc.vector.tensor_mul(
                    chunk[DH : 2 * DH, :], ctx_b[0:DH, :],
                    bc_sb[DH : 2 * DH, :],
                )
                c = len(chunks)
                chunks.append(chunk)
                for qs in range(QSUB):
                    for ob in range(OBLKS):
                        contrib_q.append((c, qs, ob))

            def emit_norm(ctx_a, ctx_b):
                rc_b, bc_ps = emit_norm_p1(ctx_a, ctx_b)
                emit_norm_p2(ctx_a, ctx_b, rc_b, bc_ps)

            # One flat, globally software-pipelined stream over (pair, kb):
            # the ctx matmuls for global step t are emitted after the score
            # matmuls for step t+1, including across pair boundaries, so the
            # PE never waits on ACT's exp and ACT never waits on pair setup.
            NSTEP = NPAIR * KBLKS
            SKEW = 2 if KBLKS >= 4 else 1
            tiles = {}
            pipe = []
            for gs in range(NSTEP + SKEW):
                cur = None
                if gs < NSTEP:
                    pr, kb = divmod(gs, KBLKS)
                    if kb == 0:
                        xq_t = xq_pool.tile([128, QB], mmdt)
                        nc.sync.dma_start(xq_t[:], xq[pr])
                        xt_t = xt_pool.tile([128, L], mmdt)
                        # split the key DMA so the first score matmuls don't
                        # wait for the whole 1MB row block
                        nc.sync.dma_start(xt_t[:, 0 : L // 4], xt[pr][:, 0 : L // 4])
                        nc.sync.dma_start(xt_t[:, L // 4 :], xt[pr][:, L // 4 :])
                        xa_a = xa_pool.tile([128, KBLKS * AUG], mmdt)
                        nc.sync.dma_start(xa_a[:], xa[2 * pr])
                        xa_b = xa_pool.tile([128, KBLKS * AUG], mmdt)
                        nc.sync.dma_start(xa_b[:], xa[2 * pr + 1])
                        nc.sync.dma_start(
                            wt_ts[pr][:], wt[:, pr * H : (pr + 1) * H]
                        )
                        ctx_a = c_psum.tile([128, QB], F32)
                        ctx_b = c_psum.tile([128, QB], F32)
                        tiles[pr] = (xt_t, xq_t, xa_a, xa_b, ctx_a, ctx_b)
                    xt_t, xq_t, xa_a, xa_b, ctx_a, ctx_b = tiles[pr]
                    # norm phase 1 allocates its PSUM slot BEFORE this step's
                    # scores tile so the scores slot-rotation parity (which
                    # pipelines scores k+2 against exp k) is preserved.
                    if KBLKS >= 4 and pr > 0:
                        if kb == 2:
                            norm_state = emit_norm_p1(
                                tiles[pr - 1][4], tiles[pr - 1][5]
                            )
                        elif kb == 3:
                            emit_norm_p2(
                                tiles[pr - 1][4], tiles[pr - 1][5], *norm_state
                            )
                    elif kb == KBLKS - 1 and pr > 0:
                        emit_norm(tiles[pr - 1][4], tiles[pr - 1][5])
                    s_ab = s_psum.tile([128, 2 * QB], F32, tag="s")
                    ksl = slice(kb * 128, (kb + 1) * 128)
                    nc.tensor.matmul(
                        s_ab[:, 0:QB], r(xt_t[0:64, ksl]),
                        r(xq_t[0:64, :]), start=True, stop=True,
                    )
                    nc.tensor.matmul(
                        s_ab[:, QB : 2 * QB], r(xt_t[64:128, ksl]),
                        r(xq_t[64:128, :]), start=True, stop=True,
                    )
                    p_ab = p_pool.tile([128, 2 * QB], mmdt, tag="p")
                    nc.scalar.activation(
                        p_ab[:], s_ab[:], mybir.ActivationFunctionType.Exp,
                        scale=SCALE,
                    )
                    cur = (gs, p_ab)
                    if kb >= 5 and contrib_q:
                        emit_contrib()
                if cur is not None:
                    pipe.append(cur)
                prev = pipe.pop(0) if (len(pipe) > SKEW or cur is None) and pipe else None
                if prev is not None:
                    gsp, pp = prev
                    prp, kbp = divmod(gsp, KBLKS)
                    _, _, xa_a, xa_b, ctx_a, ctx_b = tiles[prp]
                    asl = slice(kbp * AUG, (kbp + 1) * AUG)
                    nc.tensor.matmul(
                        ctx_a[0:AUG, :], r(xa_a[:, asl]), r(pp[:, 0:QB]),
                        start=(kbp == 0), stop=(kbp == KBLKS - 1),
                    )
                    nc.tensor.matmul(
                        ctx_b[0:AUG, :], r(xa_b[:, asl]),
                        r(pp[:, QB : 2 * QB]),
                        start=(kbp == 0), stop=(kbp == KBLKS - 1),
                    )

            emit_norm(tiles[NPAIR - 1][4], tiles[NPAIR - 1][5])
            i = 0
            while contrib_q:
                emit_contrib(alt=["ctx_a", "ctx_b"][i % 2])
                i += 1
            for qs in range(QSUB):
                nc.sync.dma_start(out[qs * 128 : (qs + 1) * 128, :], acc[qs][:])
    nc.compile()
    return nc


def shard_inputs(key, W_ctx, b_ctx, L=2048, NH=16, DH=64, QB=512, H=1024):
    """Host-side prep of per-core input dicts."""
    KBLKS = L // 128
    NPAIR = NH // 2
    AUG = DH + 1
    HC = H // 128
    Bv = key.shape[0]
    ncores = NCORES
    qper = Bv * L // (ncores * QB)  # query blocks per batch... cores per batch
    cores_per_batch = ncores // Bv

    key = np.asarray(key, dtype=np.float32)
    xh = key.reshape(Bv, L, NH, DH)
    # xt: [B, NPAIR, 128, L], pair p rows 0:64 = head 2p, 64:128 = head 2p+1
    xt_full = np.ascontiguousarray(
        xh.transpose(0, 2, 3, 1).reshape(Bv, NPAIR, 2 * DH, L)
    )
    # xa: [B, NH, 128, KBLKS*AUG] with ones in column kb*AUG+DH
    xa_full = np.empty((Bv, NH, 128, KBLKS * AUG), dtype=np.float32)
    xa_view = xa_full.reshape(Bv, NH, 128, KBLKS, AUG)
    xa_view[..., DH] = 1.0
    # x natural per head, kb-blocked: [B, NH, KB, 128, DH] -> [B, NH, 128, KB, DH]
    xa_view[..., 0:DH] = xh.reshape(Bv, KBLKS, 128, NH, DH).transpose(
        0, 3, 2, 1, 4
    )
    wt_host = np.ascontiguousarray(
        np.asarray(W_ctx, np.float32).T.reshape(HC, 128, H).transpose(1, 0, 2)
        .reshape(128, HC * H)
    )
    bias_host = np.ascontiguousarray(np.asarray(b_ctx, np.float32).reshape(1, H))
    ones_host = np.ones((1, 128), dtype=np.float32)

    in_maps = []
    meta = []
    for c in range(ncores):
        b = c // cores_per_batch
        qlo = (c % cores_per_batch) * QB
        in_maps.append(
            {
                "xt": xt_full[b],
                "xq": np.ascontiguousarray(xt_full[b][:, :, qlo : qlo + QB]),
                "xa": xa_full[b],
                "wt": wt_host,
                "bias": bias_host,
                "ones": ones_host,
            }
        )
        meta.append((b, qlo))
    return in_maps, meta


_NC_CACHE = {}


def kernel(key, W_ctx, b_ctx):
    from concourse.bass_utils import run_bass_kernel_spmd

    key = np.asarray(key, dtype=np.float32)
    if "nc" not in _NC_CACHE:
        _NC_CACHE["nc"] = build_nc(L=L, NH=NH, DH=DH, QB=512, H=H)
    nc = _NC_CACHE["nc"]
    in_maps, meta = shard_inputs(key, W_ctx, b_ctx, L=L, NH=NH, DH=DH, QB=512, H=H)
    res = run_bass_kernel_spmd(nc, in_maps, list(range(NCORES)))
    outf = np.empty((B, L, H), dtype=np.float32)
    for c, (b, qlo) in enumerate(meta):
        outf[b, qlo : qlo + 512] = res.results[c]["out"]
    return outf

